# revision 25
# baseline (speedup 1.0000x reference)
"""Trainium2 Bass kernel for an equivariant GNN message-passing layer.

Full inputs in, full output out. 8-way owner-computes sharding by edge target
node (col). The host sorts each core's edges by target window, pads tiles to
128 lanes, and pre-gathers the raw endpoint embedding rows into sequential
slabs (srcT/dstT, transposed [ch, edge]); per-edge squared distances are
precomputed on host. The device computes, per core c (nodes [c*S, (c+1)*S)):

  msg[e]  = relu(src_e @ W1 + dst_e @ W2 + dist_e * w_d + b)   (f32 PSUM)
  aggrT   = one-hot scatter-sum of msg by col                  [128, S_pad]
  outT    = Wres^T emb^T + relu(Wu1^T emb^T + Wu2^T aggrT + b_upd)

with W1 = W_msg[:128], W2 = W_msg[128:256], w_d = W_msg[256]. All matmuls are
bf16 inputs with f32 PSUM accumulation. dist+bias enter via a single rank-9
matmul per 512 edge-columns (8 block-diagonal dist rows + a ones row streaming
[w_d blocks; b tiled]). The output is produced transposed [128, S_pad]; the
host transposes back.
"""

import sys

for _p in ("/opt/trn_rl_repo",):
    if _p not in sys.path:
        sys.path.insert(0, _p)

import numpy as np
import ml_dtypes

import concourse.bacc as bacc
import concourse.bass as bass
import concourse.mybir as mybir
import concourse.tile as tile
from concourse.bass_utils import run_bass_kernel_spmd

F32 = mybir.dt.float32
BF16 = mybir.dt.bfloat16
BF = ml_dtypes.bfloat16

H = 128          # hidden/in channels (hardcoded for this problem)
RMAX = 8         # tiles per run


# --------------------------------------------------------------------------
# host-side prep
# --------------------------------------------------------------------------

def host_prep(node_embed, node_pos, W_res, W_msg, b_msg, W_upd, b_upd,
              edge_index, n_cores):
    N, C_in = node_embed.shape
    assert C_in == H and W_msg.shape == (2 * H + 1, H)
    assert N % n_cores == 0
    S = N // n_cores
    n_win = -(-S // 128)
    S_pad = n_win * 128

    row = np.asarray(edge_index[0], dtype=np.int64)
    col = np.asarray(edge_index[1], dtype=np.int64)
    pos = np.asarray(node_pos, dtype=np.float32)
    diff = pos[row] - pos[col]
    dist = np.sum(diff * diff, axis=1).astype(np.float32)   # [E]

    # global 128-node blocks, assigned to (core, slot) by sorted edge count
    # round-robin so the per-slot max across cores (which sets the padded
    # tile count) tracks the mean instead of the tail
    NB = -(-N // 128)
    assert n_cores * n_win >= NB
    blk_of_edge = col // 128
    cnt_g = np.bincount(blk_of_edge, minlength=NB)
    order = np.argsort(-cnt_g, kind="stable")
    blk_at = np.full((n_cores, n_win), -1, dtype=np.int64)  # (c, slot) -> blk
    core_of_blk = np.zeros(NB, dtype=np.int64)
    slot_of_blk = np.zeros(NB, dtype=np.int64)
    for j in range(n_win):
        for c in range(n_cores):
            i = j * n_cores + c
            if i < NB:
                g = order[i]
                blk_at[c, j] = g
                core_of_blk[g] = c
                slot_of_blk[g] = j

    core_of = core_of_blk[blk_of_edge]

    # per-core edge lists sorted by slot
    per_core = []
    counts = np.zeros((n_cores, n_win), dtype=np.int64)
    for c in range(n_cores):
        sel = np.nonzero(core_of == c)[0]
        w = slot_of_blk[blk_of_edge[sel]]
        order_e = np.argsort(w, kind="stable")
        sel, w = sel[order_e], w[order_e]
        cw = (col[sel] % 128).astype(np.float32)
        np.add.at(counts[c], w, 1)
        per_core.append((sel, cw))

    tiles_w = -(-counts.max(axis=0) // 128)             # [n_win]
    win_of_tile = []
    for w in range(n_win):
        win_of_tile += [w] * int(tiles_w[w])
    T = len(win_of_tile)
    R = -(-T // RMAX)
    T_pad = R * RMAX
    first_wp, last_wp = {}, {}
    for t, w in enumerate(win_of_tile):
        first_wp.setdefault(w, t)
        last_wp[w] = t
    tile_base = {}
    b = 0
    for w in range(n_win):
        tile_base[w] = b
        b += int(tiles_w[w])

    embT = np.ascontiguousarray(np.asarray(node_embed, dtype=np.float32).T
                                ).astype(BF)            # [H, N]

    in_maps = []
    iota = np.arange(128, dtype=np.float32)
    W_msg = np.asarray(W_msg, dtype=np.float32)
    W_upd = np.asarray(W_upd, dtype=np.float32)
    # rank-5 dist+bias rhs per half-run: rows 0..3 block-diagonal w_d,
    # row 4 = b tiled; replicated at the 4 row-group partition bases
    wdiag = np.zeros((96 + 5, 4 * 128), dtype=np.float32)
    for q in range(4):
        for k in range(4):
            wdiag[32 * q + k, k * 128:(k + 1) * 128] = W_msg[2 * H]
            wdiag[32 * q + 4, k * 128:(k + 1) * 128] = \
                np.asarray(b_msg, np.float32)
    repl = {
        "W1": np.ascontiguousarray(W_msg[:H]).astype(BF),
        "W2": np.ascontiguousarray(W_msg[H:2 * H]).astype(BF),
        "wdiag": wdiag.astype(BF),
        "W_res": np.asarray(W_res, dtype=np.float32).astype(BF),
        "Wu1": np.ascontiguousarray(W_upd[:H]).astype(BF),
        "Wu2": np.ascontiguousarray(W_upd[H:]).astype(BF),
        "bupd_col": np.asarray(b_upd, dtype=np.float32).reshape(H, 1),
        "iota_rep": np.tile(iota.reshape(1, 128), (128, 1)).astype(BF),
    }

    emb = np.asarray(node_embed, dtype=np.float32)
    for c in range(n_cores):
        sel, cw = per_core[c]
        # per-tile edge slot assignment (window-major, padded per window)
        rows_pad = np.zeros(T_pad * 128, dtype=np.int64)
        cols_pad = np.zeros(T_pad * 128, dtype=np.int64)
        valid = np.zeros(T_pad * 128, dtype=bool)
        colp = np.full((128, T_pad), -1.0, dtype=np.float32)
        distp = np.zeros((T_pad, 128), dtype=np.float32)
        start = 0
        for w in range(n_win):
            cnt = int(counts[c, w])
            if cnt:
                idx = np.arange(cnt)
                slot = (tile_base[w] + idx // 128) * 128 + idx % 128
                e = sel[start:start + cnt]
                rows_pad[slot] = row[e]
                cols_pad[slot] = col[e]
                valid[slot] = True
                distp.reshape(-1)[slot] = dist[e]
                colp[idx % 128, tile_base[w] + idx // 128] = cw[start:start + cnt]
                start += cnt
        # pre-gathered transposed slabs [ch, T_pad*128]
        srcT = embT[:, rows_pad].copy()
        dstT = embT[:, cols_pad].copy()
        srcT[:, ~valid] = 0
        dstT[:, ~valid] = 0
        # distR4: per half-run (4 tiles) [5, 128] block at partition base
        # 32*(hr%4): rows 0..3 = tile dists, row 4 = ones. Four bases so 4
        # consecutive dist matmuls land in distinct PE row groups.
        HR_pad = 2 * R
        distR = np.zeros((96 + 5, HR_pad * 128), dtype=np.float32)
        dh = distp.reshape(HR_pad, 4, 128)
        for hr in range(HR_pad):
            q = hr % 4
            for k in range(4):
                distR[32 * q + k, hr * 128:(hr + 1) * 128] = dh[hr, k]
            distR[32 * q + 4, hr * 128:(hr + 1) * 128] = 1.0
        m = dict(repl)
        shardT = np.zeros((H, S_pad), dtype=BF)
        for j in range(n_win):
            g = blk_at[c, j]
            if g >= 0:
                nb = min(128, N - g * 128)
                shardT[:, j * 128:j * 128 + nb] = \
                    emb[g * 128:g * 128 + nb].T.astype(BF)
        m["emb_shardT"] = shardT
        m["srcT"] = srcT
        m["dstT"] = dstT
        m["distR"] = distR.astype(BF)
        m["colp"] = colp.astype(BF)
        in_maps.append(m)

    cfg = dict(N=N, S=S, S_pad=S_pad, n_win=n_win, R=R, T=T, T_pad=T_pad,
               win_of_tile=win_of_tile, first_wp=first_wp, last_wp=last_wp,
               n_cores=n_cores, blk_at=blk_at)
    return cfg, in_maps


def unshard(cfg, outs):
    """Assemble the full [N, H] output from per-core [H, S_pad] transposed
    slabs laid out in (core, slot) block order."""
    N, n_win, n_cores = cfg["N"], cfg["n_win"], cfg["n_cores"]
    blk_at = cfg["blk_at"]
    out = np.empty((N, H), dtype=np.float32)
    for c in range(n_cores):
        for j in range(n_win):
            g = blk_at[c, j]
            if g >= 0:
                nb = min(128, N - g * 128)
                out[g * 128:g * 128 + nb] = \
                    outs[c][:, j * 128:j * 128 + nb].T
    return out


# --------------------------------------------------------------------------
# device program
# --------------------------------------------------------------------------

def build_program(cfg, debug=False):
    S_pad, n_win, R, T, T_pad = (cfg["S_pad"], cfg["n_win"], cfg["R"],
                                 cfg["T"], cfg["T_pad"])
    win_of_tile = cfg["win_of_tile"]
    first_wp, last_wp = cfg["first_wp"], cfg["last_wp"]

    nc = bacc.Bacc("TRN2", target_bir_lowering=False, debug=debug,
                   num_devices=cfg["n_cores"])

    HR = -(-T // 4)
    HR_pad = 2 * R

    din = lambda n, s, dt: nc.dram_tensor(n, s, dt, kind="ExternalInput")
    W1 = din("W1", [H, H], BF16)
    W2 = din("W2", [H, H], BF16)
    wdiag = din("wdiag", [96 + 5, 4 * 128], BF16)
    W_res = din("W_res", [H, H], BF16)
    Wu1 = din("Wu1", [H, H], BF16)
    Wu2 = din("Wu2", [H, H], BF16)
    bupd_col = din("bupd_col", [H, 1], F32)
    iota_rep = din("iota_rep", [128, 128], BF16)
    emb_shardT = din("emb_shardT", [H, S_pad], BF16)
    srcT = din("srcT", [H, T_pad * 128], BF16)
    dstT = din("dstT", [H, T_pad * 128], BF16)
    distR = din("distR", [96 + 5, HR_pad * 128], BF16)
    colp = din("colp", [128, T_pad], BF16)

    out_d = nc.dram_tensor("out", [H, S_pad], F32, kind="ExternalOutput")

    with tile.TileContext(nc) as tc:
        with (
            tc.tile_pool(name="const", bufs=1) as cp,
            tc.tile_pool(name="sb", bufs=2) as sb,
            tc.tile_pool(name="big", bufs=1) as bigp,
            tc.tile_pool(name="ps", bufs=2, space="PSUM") as ps,
            tc.tile_pool(name="aggp", bufs=2, space="PSUM") as aggp,
            tc.tile_pool(name="p3ps", bufs=2, space="PSUM") as p3ps,
        ):
            def cload(t, shape, dt, eng=None):
                s = cp.tile(shape, dt, tag=t.name)
                (eng or nc.sync).dma_start(s[:], t[:])
                return s

            # sync queue: W1/W2 then edge slabs immediately; scalar queue:
            # dist/one-hot consts first, P3-only consts after (needed late)
            W1s = cload(W1, [H, H], BF16)
            W2s = cload(W2, [H, H], BF16)
            wdiags = cload(wdiag, [96 + 5, 4 * 128], BF16, nc.scalar)
            distRs = cload(distR, [96 + 5, HR_pad * 128], BF16, nc.scalar)
            iotars = cload(iota_rep, [128, 128], BF16, nc.scalar)
            colps = cload(colp, [128, T_pad], BF16, nc.scalar)
            Wress = cload(W_res, [H, H], BF16, nc.scalar)
            Wu1s = cload(Wu1, [H, H], BF16, nc.scalar)
            Wu2s = cload(Wu2, [H, H], BF16, nc.scalar)
            bupds = cload(bupd_col, [H, 1], F32, nc.scalar)
            emb_sb = bigp.tile([H, S_pad], BF16, tag="emb_sb")
            nc.scalar.dma_start(emb_sb[:], emb_shardT[:])
            aggrT = bigp.tile([128, S_pad], BF16, tag="aggrT")

            # ---------- node update MLP, one 512-col block -----------------
            def emit_p3_block(b0):
                nb = min(512, S_pad - b0)
                ps_u = p3ps.tile([128, 512], F32, tag="p3ps", name="ps_u")
                pu = ps_u[:]
                nc.tensor.matmul(out=pu[:, 0:nb], lhsT=Wu1s[:],
                                 rhs=emb_sb[:, b0:b0 + nb], start=True,
                                 stop=False)
                nc.tensor.matmul(out=pu[:, 0:nb], lhsT=Wu2s[:],
                                 rhs=aggrT[:, b0:b0 + nb], start=False,
                                 stop=True)
                r_sb = sb.tile([128, 512], F32, tag="p3r", name="r_sb")
                nc.scalar.activation(out=r_sb[:, 0:nb], in_=pu[:, 0:nb],
                                     func=mybir.ActivationFunctionType.Relu,
                                     bias=bupds[:])
                ps_r = p3ps.tile([128, 512], F32, tag="p3ps", name="ps_r")
                pr = ps_r[:]
                nc.tensor.matmul(out=pr[:, 0:nb], lhsT=Wress[:],
                                 rhs=emb_sb[:, b0:b0 + nb], start=True,
                                 stop=True)
                o_sb = sb.tile([128, 512], F32, tag="p3o", name="o_sb")
                nc.vector.tensor_tensor(out=o_sb[:, 0:nb], in0=r_sb[:, 0:nb],
                                        in1=pr[:, 0:nb],
                                        op=mybir.AluOpType.add)
                nc.scalar.dma_start(out_d[:, b0:b0 + nb], o_sb[:, 0:nb])

            # window w's aggregate is final after its last tile; map final
            # tiles -> ready P3 blocks
            blocks_after = {}
            for b0 in range(0, S_pad, 512):
                wins = range(b0 // 128, min(b0 + 512, S_pad) // 128)
                fins = [last_wp[w] for w in wins if w in last_wp]
                if fins:
                    blocks_after.setdefault(max(fins), []).append(b0)
            p3_emitted = set()

            # zero windows that never receive edges, before any P3 block runs
            for w in range(n_win):
                if w not in first_wp:
                    nc.vector.memset(aggrT[:, w * 128:(w + 1) * 128], 0.0)

            # ---------- edge loop ------------------------------------------
            # segsum for run ri is emitted after run ri+1's projection
            # matmuls so the PE never stalls on relu (ACT) / o8 (DVE)
            aggr_tiles = {}
            copy_alt = [0]

            def emit_segsum(t0, L, msg_bf, o8):
                for k in range(L):
                    t = t0 + k
                    w = win_of_tile[t]
                    if t == first_wp[w]:
                        aggr_t = aggp.tile([128, H], F32, tag="aggr")
                        aggr_tiles[w] = aggr_t
                    nc.tensor.matmul(out=aggr_tiles[w][:],
                                     lhsT=msg_bf[:, k, :], rhs=o8[:, k, :],
                                     start=(t == first_wp[w]),
                                     stop=(t == last_wp[w]))
                    if t == last_wp[w]:
                        dstw = aggrT[:, w * 128:(w + 1) * 128]
                        if copy_alt[0] % 2:
                            nc.scalar.activation(
                                out=dstw, in_=aggr_tiles[w][:],
                                func=mybir.ActivationFunctionType.Copy)
                        else:
                            nc.vector.tensor_scalar_add(
                                out=dstw, in0=aggr_tiles[w][:], scalar1=0.0)
                        copy_alt[0] += 1
                        del aggr_tiles[w]
                        for b0 in blocks_after.get(t, []):
                            emit_p3_block(b0)
                            p3_emitted.add(b0)

            # half-runs of 4 tiles; dist matmuls batched 4-at-a-time into
            # the 4 PE row groups (distinct tile_position -> concurrent)
            slabs = {}
            msg_pss = {}

            def emit_dist(hr):
                q = hr % 4
                msg_ps = ps.tile([128, 4, H], F32, tag="msgps")
                msg_pss[hr] = msg_ps
                flat = msg_ps[:].rearrange("p k e -> p (k e)")
                nc.tensor.matmul(out=flat[:, 0:512],
                                 lhsT=distRs[32 * q:32 * q + 5,
                                             hr * 128:(hr + 1) * 128],
                                 rhs=wdiags[32 * q:32 * q + 5, :],
                                 start=True, stop=False,
                                 tile_position=(32 * q, 0),
                                 skip_group_check=True)

            pending = None
            emit_dist(0)
            for hr in range(HR):
                t0 = hr * 4
                L = min(4, T - t0)
                if hr % 2 == 0:
                    ri = hr // 2
                    src_sb = sb.tile([128, RMAX, H], BF16, tag="src", bufs=4)
                    nc.sync.dma_start(
                        src_sb[:], srcT[:, ri * 1024:(ri + 1) * 1024]
                        .rearrange("p (k e) -> p k e", k=RMAX))
                    dst_sb = sb.tile([128, RMAX, H], BF16, tag="dst", bufs=4)
                    nc.gpsimd.dma_start(
                        dst_sb[:], dstT[:, ri * 1024:(ri + 1) * 1024]
                        .rearrange("p (k e) -> p k e", k=RMAX))
                    slabs[hr] = slabs[hr + 1] = (src_sb, dst_sb)
                # dist quad one half-run ahead of its projections
                if hr % 4 == 1:
                    for h2 in range(hr, min(hr + 4, HR)):
                        emit_dist(h2)
                src_sb, dst_sb = slabs.pop(hr)
                msg_ps = msg_pss.pop(hr)
                ko = (hr % 2) * 4
                # + src @ W1 + dst @ W2 per tile
                for k in range(L):
                    nc.tensor.matmul(out=msg_ps[:, k, :],
                                     lhsT=src_sb[:, ko + k, :], rhs=W1s[:],
                                     start=False, stop=False,
                                     skip_group_check=True)
                    nc.tensor.matmul(out=msg_ps[:, k, :],
                                     lhsT=dst_sb[:, ko + k, :], rhs=W2s[:],
                                     start=False, stop=True,
                                     skip_group_check=True)
                # previous half-run's segment-sum (PE never stalls on relu)
                if pending is not None:
                    emit_segsum(*pending)
                # relu + cast (ACT)
                msg_bf = sb.tile([128, 4, H], BF16, tag="msgb", bufs=4)
                nc.scalar.activation(out=msg_bf[:, 0:L, :],
                                     in_=msg_ps[:, 0:L, :],
                                     func=mybir.ActivationFunctionType.Relu)
                # scatter one-hot by local col
                o8 = sb.tile([128, 4, H], BF16, tag="o8", bufs=4)
                nc.vector.tensor_tensor(
                    out=o8[:, 0:L, :],
                    in0=colps[:, t0:t0 + L, None].to_broadcast([128, L, 128]),
                    in1=iotars[:, None, :].to_broadcast([128, L, 128]),
                    op=mybir.AluOpType.is_equal)
                pending = (t0, L, msg_bf, o8)

            if pending is not None:
                emit_segsum(*pending)
            for b0 in range(0, S_pad, 512):
                if b0 not in p3_emitted:
                    emit_p3_block(b0)

    nc.compile()
    return nc


# --------------------------------------------------------------------------
# entry point
# --------------------------------------------------------------------------

def kernel(node_embed, node_pos, W_res, W_msg, b_msg, W_upd, b_upd,
           edge_index, n_cores=8, _run=None):
    cfg, in_maps = host_prep(node_embed, node_pos, W_res, W_msg, b_msg,
                             W_upd, b_upd, edge_index, n_cores)
    nc = build_program(cfg)
    if _run is None:
        res = run_bass_kernel_spmd(nc, in_maps, core_ids=list(range(n_cores)))
        outs = [res.results[c]["out"] for c in range(n_cores)]
    else:
        outs = _run(nc, in_maps)
    return unshard(cfg, outs)


# revision 32
# speedup vs baseline: 1.5934x; 1.5934x over previous
"""Trainium2 Bass kernel for an equivariant GNN message-passing layer.

Full inputs in, full output out. 8-way owner-computes sharding by edge target
node (col). The host sorts each core's edges by target window, pads tiles to
128 lanes, and pre-gathers the raw endpoint embedding rows into sequential
slabs (srcT/dstT, transposed [ch, edge]); per-edge squared distances are
precomputed on host. The device computes, per core c (nodes [c*S, (c+1)*S)):

  msg[e]  = relu(src_e @ W1 + dst_e @ W2 + dist_e * w_d + b)   (f32 PSUM)
  aggrT   = one-hot scatter-sum of msg by col                  [128, S_pad]
  outT    = Wres^T emb^T + relu(Wu1^T emb^T + Wu2^T aggrT + b_upd)

with W1 = W_msg[:128], W2 = W_msg[128:256], w_d = W_msg[256]. All matmuls are
bf16 inputs with f32 PSUM accumulation. dist+bias enter via a single rank-9
matmul per 512 edge-columns (8 block-diagonal dist rows + a ones row streaming
[w_d blocks; b tiled]). The output is produced transposed [128, S_pad]; the
host transposes back.
"""

import sys

for _p in ("/opt/trn_rl_repo",):
    if _p not in sys.path:
        sys.path.insert(0, _p)

import numpy as np
import ml_dtypes

import concourse.bacc as bacc
import concourse.bass as bass
import concourse.mybir as mybir
import concourse.tile as tile
from concourse.bass_utils import run_bass_kernel_spmd

F32 = mybir.dt.float32
BF16 = mybir.dt.bfloat16
BF = ml_dtypes.bfloat16

H = 128          # hidden/in channels (hardcoded for this problem)
RMAX = 8         # tiles per run


# --------------------------------------------------------------------------
# host-side prep
# --------------------------------------------------------------------------

def host_prep(node_embed, node_pos, W_res, W_msg, b_msg, W_upd, b_upd,
              edge_index, n_cores):
    N, C_in = node_embed.shape
    assert C_in == H and W_msg.shape == (2 * H + 1, H)
    assert N % n_cores == 0
    S = N // n_cores
    n_win = -(-S // 128)
    S_pad = n_win * 128

    row = np.asarray(edge_index[0], dtype=np.int64)
    col = np.asarray(edge_index[1], dtype=np.int64)
    pos = np.asarray(node_pos, dtype=np.float32)
    diff = pos[row] - pos[col]
    dist = np.sum(diff * diff, axis=1).astype(np.float32)   # [E]

    # global 128-node blocks, assigned to (core, slot) by sorted edge count
    # round-robin so the per-slot max across cores (which sets the padded
    # tile count) tracks the mean instead of the tail
    NB = -(-N // 128)
    assert n_cores * n_win >= NB
    blk_of_edge = col // 128
    cnt_g = np.bincount(blk_of_edge, minlength=NB)
    order = np.argsort(-cnt_g, kind="stable")
    blk_at = np.full((n_cores, n_win), -1, dtype=np.int64)  # (c, slot) -> blk
    core_of_blk = np.zeros(NB, dtype=np.int64)
    slot_of_blk = np.zeros(NB, dtype=np.int64)
    for j in range(n_win):
        for c in range(n_cores):
            i = j * n_cores + c
            if i < NB:
                g = order[i]
                blk_at[c, j] = g
                core_of_blk[g] = c
                slot_of_blk[g] = j

    core_of = core_of_blk[blk_of_edge]

    # per-core edge lists sorted by slot
    per_core = []
    counts = np.zeros((n_cores, n_win), dtype=np.int64)
    for c in range(n_cores):
        sel = np.nonzero(core_of == c)[0]
        w = slot_of_blk[blk_of_edge[sel]]
        order_e = np.argsort(w, kind="stable")
        sel, w = sel[order_e], w[order_e]
        cw = (col[sel] % 128).astype(np.float32)
        np.add.at(counts[c], w, 1)
        per_core.append((sel, cw))

    tiles_w = -(-counts.max(axis=0) // 128)             # [n_win]
    win_of_tile = []
    for w in range(n_win):
        win_of_tile += [w] * int(tiles_w[w])
    T = len(win_of_tile)
    R = -(-T // RMAX)
    T_pad = R * RMAX
    first_wp, last_wp = {}, {}
    for t, w in enumerate(win_of_tile):
        first_wp.setdefault(w, t)
        last_wp[w] = t
    tile_base = {}
    b = 0
    for w in range(n_win):
        tile_base[w] = b
        b += int(tiles_w[w])

    embT = np.ascontiguousarray(np.asarray(node_embed, dtype=np.float32).T
                                ).astype(BF)            # [H, N]

    in_maps = []
    iota = np.arange(128, dtype=np.float32)
    W_msg = np.asarray(W_msg, dtype=np.float32)
    W_upd = np.asarray(W_upd, dtype=np.float32)
    # rank-9 dist+bias rhs: rows 0..7 block-diagonal w_d, row 8 = b tiled;
    # duplicated at partition base 32 so the two per-run dist matmuls can
    # occupy distinct PE row groups (concurrent streams into their banks)
    wdiag = np.zeros((32 + RMAX + 1, RMAX * 128), dtype=np.float32)
    for k in range(RMAX):
        wdiag[k, k * 128:(k + 1) * 128] = W_msg[2 * H]
        wdiag[RMAX, k * 128:(k + 1) * 128] = np.asarray(b_msg, np.float32)
    wdiag[32:32 + RMAX + 1] = wdiag[:RMAX + 1]
    repl = {
        "W1": np.ascontiguousarray(W_msg[:H]).astype(BF),
        "W2": np.ascontiguousarray(W_msg[H:2 * H]).astype(BF),
        "wdiag": wdiag.astype(BF),
        "W_res": np.asarray(W_res, dtype=np.float32).astype(BF),
        "Wu1": np.ascontiguousarray(W_upd[:H]).astype(BF),
        "Wu2": np.ascontiguousarray(W_upd[H:]).astype(BF),
        "bupd_col": np.asarray(b_upd, dtype=np.float32).reshape(H, 1),
        "iota_rep": np.tile(iota.reshape(1, 128), (128, 1)).astype(BF),
    }

    emb = np.asarray(node_embed, dtype=np.float32)
    for c in range(n_cores):
        sel, cw = per_core[c]
        # per-tile edge slot assignment (window-major, padded per window)
        rows_pad = np.zeros(T_pad * 128, dtype=np.int64)
        cols_pad = np.zeros(T_pad * 128, dtype=np.int64)
        valid = np.zeros(T_pad * 128, dtype=bool)
        colp = np.full((128, T_pad), -1.0, dtype=np.float32)
        distp = np.zeros((T_pad, 128), dtype=np.float32)
        start = 0
        for w in range(n_win):
            cnt = int(counts[c, w])
            if cnt:
                idx = np.arange(cnt)
                slot = (tile_base[w] + idx // 128) * 128 + idx % 128
                e = sel[start:start + cnt]
                rows_pad[slot] = row[e]
                cols_pad[slot] = col[e]
                valid[slot] = True
                distp.reshape(-1)[slot] = dist[e]
                colp[idx % 128, tile_base[w] + idx // 128] = cw[start:start + cnt]
                start += cnt
        # pre-gathered transposed slabs [ch, T_pad*128]
        srcT = embT[:, rows_pad].copy()
        dstT = embT[:, cols_pad].copy()
        srcT[:, ~valid] = 0
        dstT[:, ~valid] = 0
        # distR: per run [9, 128]: rows 0..7 = tile dists, row 8 = ones;
        # duplicated at partition base 32 (see wdiag)
        distR = np.zeros((32 + RMAX + 1, R * 128), dtype=np.float32)
        dr = distp.reshape(R, RMAX, 128)
        for k in range(RMAX):
            distR[k] = dr[:, k, :].reshape(R * 128)
        distR[RMAX] = 1.0
        distR[32:32 + RMAX + 1] = distR[:RMAX + 1]
        m = dict(repl)
        shardT = np.zeros((H, S_pad), dtype=BF)
        for j in range(n_win):
            g = blk_at[c, j]
            if g >= 0:
                nb = min(128, N - g * 128)
                shardT[:, j * 128:j * 128 + nb] = \
                    emb[g * 128:g * 128 + nb].T.astype(BF)
        m["emb_shardT"] = shardT
        m["srcT"] = srcT
        m["dstT"] = dstT
        m["distR"] = distR.astype(BF)
        m["colp"] = colp.astype(BF)
        in_maps.append(m)

    cfg = dict(N=N, S=S, S_pad=S_pad, n_win=n_win, R=R, T=T, T_pad=T_pad,
               win_of_tile=win_of_tile, first_wp=first_wp, last_wp=last_wp,
               n_cores=n_cores, blk_at=blk_at)
    return cfg, in_maps


def unshard(cfg, outs):
    """Assemble the full [N, H] output from per-core [H, S_pad] transposed
    slabs laid out in (core, slot) block order."""
    N, n_win, n_cores = cfg["N"], cfg["n_win"], cfg["n_cores"]
    blk_at = cfg["blk_at"]
    out = np.empty((N, H), dtype=np.float32)
    for c in range(n_cores):
        for j in range(n_win):
            g = blk_at[c, j]
            if g >= 0:
                nb = min(128, N - g * 128)
                out[g * 128:g * 128 + nb] = \
                    outs[c][:, j * 128:j * 128 + nb].T
    return out


# --------------------------------------------------------------------------
# device program
# --------------------------------------------------------------------------

def build_program(cfg, debug=False):
    S_pad, n_win, R, T, T_pad = (cfg["S_pad"], cfg["n_win"], cfg["R"],
                                 cfg["T"], cfg["T_pad"])
    win_of_tile = cfg["win_of_tile"]
    first_wp, last_wp = cfg["first_wp"], cfg["last_wp"]

    nc = bacc.Bacc("TRN2", target_bir_lowering=False, debug=debug,
                   num_devices=cfg["n_cores"])

    din = lambda n, s, dt: nc.dram_tensor(n, s, dt, kind="ExternalInput")
    W1 = din("W1", [H, H], BF16)
    W2 = din("W2", [H, H], BF16)
    wdiag = din("wdiag", [32 + RMAX + 1, RMAX * 128], BF16)
    W_res = din("W_res", [H, H], BF16)
    Wu1 = din("Wu1", [H, H], BF16)
    Wu2 = din("Wu2", [H, H], BF16)
    bupd_col = din("bupd_col", [H, 1], F32)
    iota_rep = din("iota_rep", [128, 128], BF16)
    emb_shardT = din("emb_shardT", [H, S_pad], BF16)
    srcT = din("srcT", [H, T_pad * 128], BF16)
    dstT = din("dstT", [H, T_pad * 128], BF16)
    distR = din("distR", [32 + RMAX + 1, R * 128], BF16)
    colp = din("colp", [128, T_pad], BF16)

    out_d = nc.dram_tensor("out", [H, S_pad], F32, kind="ExternalOutput")

    with tile.TileContext(nc) as tc:
        with (
            tc.tile_pool(name="const", bufs=1) as cp,
            tc.tile_pool(name="sb", bufs=2) as sb,
            tc.tile_pool(name="big", bufs=1) as bigp,
            tc.tile_pool(name="ps", bufs=2, space="PSUM") as ps,
            tc.tile_pool(name="aggp", bufs=2, space="PSUM") as aggp,
            tc.tile_pool(name="p3ps", bufs=2, space="PSUM") as p3ps,
        ):
            def cload(t, shape, dt, eng=None):
                s = cp.tile(shape, dt, tag=t.name)
                (eng or nc.sync).dma_start(s[:], t[:])
                return s

            # sync queue: W1/W2 then edge slabs immediately; scalar queue:
            # dist/one-hot consts first, P3-only consts after (needed late)
            W1s = cload(W1, [H, H], BF16)
            W2s = cload(W2, [H, H], BF16)
            wdiags = cload(wdiag, [32 + RMAX + 1, RMAX * 128], BF16,
                           nc.scalar)
            distRs = cload(distR, [32 + RMAX + 1, R * 128], BF16, nc.scalar)
            iotars = cload(iota_rep, [128, 128], BF16, nc.scalar)
            colps = cload(colp, [128, T_pad], BF16, nc.scalar)
            Wress = cload(W_res, [H, H], BF16, nc.scalar)
            Wu1s = cload(Wu1, [H, H], BF16, nc.scalar)
            Wu2s = cload(Wu2, [H, H], BF16, nc.scalar)
            bupds = cload(bupd_col, [H, 1], F32, nc.scalar)
            emb_sb = bigp.tile([H, S_pad], BF16, tag="emb_sb")
            nc.scalar.dma_start(emb_sb[:], emb_shardT[:])
            aggrT = bigp.tile([128, S_pad], BF16, tag="aggrT")

            # ---------- node update MLP, one 512-col block -----------------
            def emit_p3_block(b0):
                nb = min(512, S_pad - b0)
                ps_u = p3ps.tile([128, 512], F32, tag="p3ps", name="ps_u")
                pu = ps_u[:]
                nc.tensor.matmul(out=pu[:, 0:nb], lhsT=Wu1s[:],
                                 rhs=emb_sb[:, b0:b0 + nb], start=True,
                                 stop=False)
                nc.tensor.matmul(out=pu[:, 0:nb], lhsT=Wu2s[:],
                                 rhs=aggrT[:, b0:b0 + nb], start=False,
                                 stop=True)
                r_sb = sb.tile([128, 512], F32, tag="p3r", name="r_sb")
                nc.scalar.activation(out=r_sb[:, 0:nb], in_=pu[:, 0:nb],
                                     func=mybir.ActivationFunctionType.Relu,
                                     bias=bupds[:])
                ps_r = p3ps.tile([128, 512], F32, tag="p3ps", name="ps_r")
                pr = ps_r[:]
                nc.tensor.matmul(out=pr[:, 0:nb], lhsT=Wress[:],
                                 rhs=emb_sb[:, b0:b0 + nb], start=True,
                                 stop=True)
                o_sb = sb.tile([128, 512], F32, tag="p3o", name="o_sb")
                nc.vector.tensor_tensor(out=o_sb[:, 0:nb], in0=r_sb[:, 0:nb],
                                        in1=pr[:, 0:nb],
                                        op=mybir.AluOpType.add)
                nc.scalar.dma_start(out_d[:, b0:b0 + nb], o_sb[:, 0:nb])

            # window w's aggregate is final after its last tile; map final
            # tiles -> ready P3 blocks
            blocks_after = {}
            for b0 in range(0, S_pad, 512):
                wins = range(b0 // 128, min(b0 + 512, S_pad) // 128)
                fins = [last_wp[w] for w in wins if w in last_wp]
                if fins:
                    blocks_after.setdefault(max(fins), []).append(b0)
            p3_emitted = set()

            # zero windows that never receive edges, before any P3 block runs
            for w in range(n_win):
                if w not in first_wp:
                    nc.vector.memset(aggrT[:, w * 128:(w + 1) * 128], 0.0)

            # ---------- edge loop ------------------------------------------
            # segsum for run ri is emitted after run ri+1's projection
            # matmuls so the PE never stalls on relu (ACT) / o8 (DVE)
            aggr_tiles = {}
            copy_alt = [0]

            def emit_segsum(t0, L, msg_bf, o8):
                for k in range(L):
                    t = t0 + k
                    w = win_of_tile[t]
                    if t == first_wp[w]:
                        aggr_t = aggp.tile([128, H], F32, tag="aggr")
                        aggr_tiles[w] = aggr_t
                    nc.tensor.matmul(out=aggr_tiles[w][:],
                                     lhsT=msg_bf[:, k, :], rhs=o8[:, k, :],
                                     start=(t == first_wp[w]),
                                     stop=(t == last_wp[w]))
                    if t == last_wp[w]:
                        dstw = aggrT[:, w * 128:(w + 1) * 128]
                        if copy_alt[0] % 2:
                            nc.scalar.activation(
                                out=dstw, in_=aggr_tiles[w][:],
                                func=mybir.ActivationFunctionType.Copy)
                        else:
                            nc.vector.tensor_scalar_add(
                                out=dstw, in0=aggr_tiles[w][:], scalar1=0.0)
                        copy_alt[0] += 1
                        del aggr_tiles[w]
                        for b0 in blocks_after.get(t, []):
                            emit_p3_block(b0)
                            p3_emitted.add(b0)

            pending = None
            for ri in range(R):
                t0 = ri * RMAX
                L = min(RMAX, T - t0)

                src_sb = sb.tile([128, RMAX, H], BF16, tag="src", bufs=4)
                nc.sync.dma_start(src_sb[:, 0:L, :],
                                  srcT[:, t0 * 128:(t0 + L) * 128]
                                  .rearrange("p (k e) -> p k e", k=L))
                dst_sb = sb.tile([128, RMAX, H], BF16, tag="dst", bufs=4)
                nc.gpsimd.dma_start(dst_sb[:, 0:L, :],
                                    dstT[:, t0 * 128:(t0 + L) * 128]
                                    .rearrange("p (k e) -> p k e", k=L))

                msg_ps = ps.tile([128, RMAX, H], F32, tag="msgps")
                flat = msg_ps[:].rearrange("p k e -> p (k e)")
                # dist*w_d + b (rank-9, block-diagonal), opens accumulation;
                # the two 512-blocks go to distinct PE row groups so their
                # streams overlap (they target different PSUM banks)
                for o in range(0, L * 128, 512):
                    oe = min(o + 512, L * 128)
                    qb = 0 if o == 0 else 32
                    nc.tensor.matmul(out=flat[:, o:oe],
                                     lhsT=distRs[qb:qb + RMAX + 1,
                                                 ri * 128:(ri + 1) * 128],
                                     rhs=wdiags[qb:qb + RMAX + 1, o:oe],
                                     start=True, stop=False,
                                     tile_position=(qb, 0),
                                     skip_group_check=True)
                # + src @ W1 + dst @ W2 per tile
                for k in range(L):
                    nc.tensor.matmul(out=msg_ps[:, k, :],
                                     lhsT=src_sb[:, k, :], rhs=W1s[:],
                                     start=False, stop=False,
                                     skip_group_check=True)
                    nc.tensor.matmul(out=msg_ps[:, k, :],
                                     lhsT=dst_sb[:, k, :], rhs=W2s[:],
                                     start=False, stop=True,
                                     skip_group_check=True)
                # previous run's segment-sum (PE waits on its relu no more)
                if pending is not None:
                    emit_segsum(*pending)
                # relu + cast (ACT)
                msg_bf = sb.tile([128, RMAX, H], BF16, tag="msgb", bufs=3)
                nc.scalar.activation(out=msg_bf[:, 0:L, :],
                                     in_=msg_ps[:, 0:L, :],
                                     func=mybir.ActivationFunctionType.Relu)
                # scatter one-hot by local col
                o8 = sb.tile([128, RMAX, H], BF16, tag="o8", bufs=3)
                nc.vector.tensor_tensor(
                    out=o8[:, 0:L, :],
                    in0=colps[:, t0:t0 + L, None].to_broadcast([128, L, 128]),
                    in1=iotars[:, None, :].to_broadcast([128, L, 128]),
                    op=mybir.AluOpType.is_equal)
                pending = (t0, L, msg_bf, o8)

            if pending is not None:
                emit_segsum(*pending)
            for b0 in range(0, S_pad, 512):
                if b0 not in p3_emitted:
                    emit_p3_block(b0)

    nc.compile()
    return nc


# --------------------------------------------------------------------------
# entry point
# --------------------------------------------------------------------------

def kernel(node_embed, node_pos, W_res, W_msg, b_msg, W_upd, b_upd,
           edge_index, n_cores=8, _run=None):
    cfg, in_maps = host_prep(node_embed, node_pos, W_res, W_msg, b_msg,
                             W_upd, b_upd, edge_index, n_cores)
    nc = build_program(cfg)
    if _run is None:
        res = run_bass_kernel_spmd(nc, in_maps, core_ids=list(range(n_cores)))
        outs = [res.results[c]["out"] for c in range(n_cores)]
    else:
        outs = _run(nc, in_maps)
    return unshard(cfg, outs)


# revision 35
# speedup vs baseline: 1.8219x; 1.1434x over previous
"""Trainium2 Bass kernel for an equivariant GNN message-passing layer.

Full inputs in, full output out. 8-way owner-computes sharding by edge target
node (col). The host sorts each core's edges by target window, pads tiles to
128 lanes, and pre-gathers the raw endpoint embedding rows into sequential
slabs (srcT/dstT, transposed [ch, edge]); per-edge squared distances are
precomputed on host. The device computes, per core c (nodes [c*S, (c+1)*S)):

  msg[e]  = relu(src_e @ W1 + dst_e @ W2 + dist_e * w_d + b)   (f32 PSUM)
  aggrT   = one-hot scatter-sum of msg by col                  [128, S_pad]
  outT    = Wres^T emb^T + relu(Wu1^T emb^T + Wu2^T aggrT + b_upd)

with W1 = W_msg[:128], W2 = W_msg[128:256], w_d = W_msg[256]. All matmuls are
bf16 inputs with f32 PSUM accumulation. dist+bias enter via a single rank-9
matmul per 512 edge-columns (8 block-diagonal dist rows + a ones row streaming
[w_d blocks; b tiled]). The output is produced transposed [128, S_pad]; the
host transposes back.
"""

import sys

for _p in ("/opt/trn_rl_repo",):
    if _p not in sys.path:
        sys.path.insert(0, _p)

import numpy as np
import ml_dtypes

import concourse.bacc as bacc
import concourse.bass as bass
import concourse.mybir as mybir
import concourse.tile as tile
from concourse.bass_utils import run_bass_kernel_spmd

F32 = mybir.dt.float32
BF16 = mybir.dt.bfloat16
BF = ml_dtypes.bfloat16

H = 128          # hidden/in channels (hardcoded for this problem)
RMAX = 8         # tiles per run


# --------------------------------------------------------------------------
# host-side prep
# --------------------------------------------------------------------------

def host_prep(node_embed, node_pos, W_res, W_msg, b_msg, W_upd, b_upd,
              edge_index, n_cores):
    N, C_in = node_embed.shape
    assert C_in == H and W_msg.shape == (2 * H + 1, H)
    assert N % n_cores == 0
    S = N // n_cores
    n_win = -(-S // 128)
    S_pad = n_win * 128

    row = np.asarray(edge_index[0], dtype=np.int64)
    col = np.asarray(edge_index[1], dtype=np.int64)
    pos = np.asarray(node_pos, dtype=np.float32)
    diff = pos[row] - pos[col]
    dist = np.sum(diff * diff, axis=1).astype(np.float32)   # [E]

    # global 128-node blocks, assigned to (core, slot) by sorted edge count
    # round-robin so the per-slot max across cores (which sets the padded
    # tile count) tracks the mean instead of the tail
    NB = -(-N // 128)
    assert n_cores * n_win >= NB
    blk_of_edge = col // 128
    cnt_g = np.bincount(blk_of_edge, minlength=NB)
    order = np.argsort(-cnt_g, kind="stable")
    blk_at = np.full((n_cores, n_win), -1, dtype=np.int64)  # (c, slot) -> blk
    core_of_blk = np.zeros(NB, dtype=np.int64)
    slot_of_blk = np.zeros(NB, dtype=np.int64)
    for j in range(n_win):
        for c in range(n_cores):
            i = j * n_cores + c
            if i < NB:
                g = order[i]
                blk_at[c, j] = g
                core_of_blk[g] = c
                slot_of_blk[g] = j

    core_of = core_of_blk[blk_of_edge]

    # per-core edge lists sorted by slot
    per_core = []
    counts = np.zeros((n_cores, n_win), dtype=np.int64)
    for c in range(n_cores):
        sel = np.nonzero(core_of == c)[0]
        w = slot_of_blk[blk_of_edge[sel]]
        order_e = np.argsort(w, kind="stable")
        sel, w = sel[order_e], w[order_e]
        cw = (col[sel] % 128).astype(np.float32)
        np.add.at(counts[c], w, 1)
        per_core.append((sel, cw))

    tiles_w = -(-counts.max(axis=0) // 128)             # [n_win]
    win_of_tile = []
    for w in range(n_win):
        win_of_tile += [w] * int(tiles_w[w])
    T = len(win_of_tile)
    R = -(-T // RMAX)
    T_pad = R * RMAX
    first_wp, last_wp = {}, {}
    for t, w in enumerate(win_of_tile):
        first_wp.setdefault(w, t)
        last_wp[w] = t
    tile_base = {}
    b = 0
    for w in range(n_win):
        tile_base[w] = b
        b += int(tiles_w[w])

    embT = np.ascontiguousarray(np.asarray(node_embed, dtype=np.float32).T
                                ).astype(BF)            # [H, N]

    in_maps = []
    iota = np.arange(128, dtype=np.float32)
    W_msg = np.asarray(W_msg, dtype=np.float32)
    W_upd = np.asarray(W_upd, dtype=np.float32)
    # rank-9 dist+bias rhs: rows 0..7 block-diagonal w_d, row 8 = b tiled
    wdiag = np.zeros((RMAX + 1, RMAX * 128), dtype=np.float32)
    for k in range(RMAX):
        wdiag[k, k * 128:(k + 1) * 128] = W_msg[2 * H]
        wdiag[RMAX, k * 128:(k + 1) * 128] = np.asarray(b_msg, np.float32)
    repl = {
        "W1": np.ascontiguousarray(W_msg[:H]).astype(BF),
        "W2": np.ascontiguousarray(W_msg[H:2 * H]).astype(BF),
        "wdiag": wdiag.astype(BF),
        "W_res": np.asarray(W_res, dtype=np.float32).astype(BF),
        "Wu1": np.ascontiguousarray(W_upd[:H]).astype(BF),
        "Wu2": np.ascontiguousarray(W_upd[H:]).astype(BF),
        "bupd_col": np.asarray(b_upd, dtype=np.float32).reshape(H, 1),
        "iota_rep": np.tile(iota.reshape(1, 128), (128, 1)).astype(BF),
    }

    emb = np.asarray(node_embed, dtype=np.float32)
    for c in range(n_cores):
        sel, cw = per_core[c]
        # per-tile edge slot assignment (window-major, padded per window)
        rows_pad = np.zeros(T_pad * 128, dtype=np.int64)
        cols_pad = np.zeros(T_pad * 128, dtype=np.int64)
        valid = np.zeros(T_pad * 128, dtype=bool)
        colp = np.full((128, T_pad), -1.0, dtype=np.float32)
        distp = np.zeros((T_pad, 128), dtype=np.float32)
        start = 0
        for w in range(n_win):
            cnt = int(counts[c, w])
            if cnt:
                idx = np.arange(cnt)
                slot = (tile_base[w] + idx // 128) * 128 + idx % 128
                e = sel[start:start + cnt]
                rows_pad[slot] = row[e]
                cols_pad[slot] = col[e]
                valid[slot] = True
                distp.reshape(-1)[slot] = dist[e]
                colp[idx % 128, tile_base[w] + idx // 128] = cw[start:start + cnt]
                start += cnt
        # pre-gathered transposed slabs [ch, T_pad*128]
        srcT = embT[:, rows_pad].copy()
        dstT = embT[:, cols_pad].copy()
        srcT[:, ~valid] = 0
        dstT[:, ~valid] = 0
        # distR: per run [9, 128]: rows 0..7 = tile dists, row 8 = ones
        distR = np.zeros((RMAX + 1, R * 128), dtype=np.float32)
        dr = distp.reshape(R, RMAX, 128)
        for k in range(RMAX):
            distR[k] = dr[:, k, :].reshape(R * 128)
        distR[RMAX] = 1.0
        m = dict(repl)
        shardT = np.zeros((H, S_pad), dtype=BF)
        for j in range(n_win):
            g = blk_at[c, j]
            if g >= 0:
                nb = min(128, N - g * 128)
                shardT[:, j * 128:j * 128 + nb] = \
                    emb[g * 128:g * 128 + nb].T.astype(BF)
        m["emb_shardT"] = shardT
        m["srcT"] = srcT
        m["dstT"] = dstT
        m["distR"] = distR.astype(BF)
        m["colp"] = colp.astype(BF)
        in_maps.append(m)

    cfg = dict(N=N, S=S, S_pad=S_pad, n_win=n_win, R=R, T=T, T_pad=T_pad,
               win_of_tile=win_of_tile, first_wp=first_wp, last_wp=last_wp,
               n_cores=n_cores, blk_at=blk_at)
    return cfg, in_maps


def unshard(cfg, outs):
    """Assemble the full [N, H] output from per-core [H, S_pad] transposed
    slabs laid out in (core, slot) block order."""
    N, n_win, n_cores = cfg["N"], cfg["n_win"], cfg["n_cores"]
    blk_at = cfg["blk_at"]
    out = np.empty((N, H), dtype=np.float32)
    for c in range(n_cores):
        for j in range(n_win):
            g = blk_at[c, j]
            if g >= 0:
                nb = min(128, N - g * 128)
                out[g * 128:g * 128 + nb] = \
                    outs[c][:, j * 128:j * 128 + nb].T
    return out


# --------------------------------------------------------------------------
# device program
# --------------------------------------------------------------------------

def build_program(cfg, debug=False):
    S_pad, n_win, R, T, T_pad = (cfg["S_pad"], cfg["n_win"], cfg["R"],
                                 cfg["T"], cfg["T_pad"])
    win_of_tile = cfg["win_of_tile"]
    first_wp, last_wp = cfg["first_wp"], cfg["last_wp"]

    nc = bacc.Bacc("TRN2", target_bir_lowering=False, debug=debug,
                   num_devices=cfg["n_cores"])

    din = lambda n, s, dt: nc.dram_tensor(n, s, dt, kind="ExternalInput")
    W1 = din("W1", [H, H], BF16)
    W2 = din("W2", [H, H], BF16)
    wdiag = din("wdiag", [RMAX + 1, RMAX * 128], BF16)
    W_res = din("W_res", [H, H], BF16)
    Wu1 = din("Wu1", [H, H], BF16)
    Wu2 = din("Wu2", [H, H], BF16)
    bupd_col = din("bupd_col", [H, 1], F32)
    iota_rep = din("iota_rep", [128, 128], BF16)
    emb_shardT = din("emb_shardT", [H, S_pad], BF16)
    srcT = din("srcT", [H, T_pad * 128], BF16)
    dstT = din("dstT", [H, T_pad * 128], BF16)
    distR = din("distR", [RMAX + 1, R * 128], BF16)
    colp = din("colp", [128, T_pad], BF16)

    out_d = nc.dram_tensor("out", [H, S_pad], F32, kind="ExternalOutput")

    with tile.TileContext(nc) as tc:
        with (
            tc.tile_pool(name="const", bufs=1) as cp,
            tc.tile_pool(name="sb", bufs=2) as sb,
            tc.tile_pool(name="big", bufs=1) as bigp,
            tc.tile_pool(name="ps", bufs=2, space="PSUM") as ps,
            tc.tile_pool(name="aggp", bufs=2, space="PSUM") as aggp,
            tc.tile_pool(name="p3ps", bufs=2, space="PSUM") as p3ps,
        ):
            def cload(t, shape, dt, eng=None):
                s = cp.tile(shape, dt, tag=t.name)
                (eng or nc.sync).dma_start(s[:], t[:])
                return s

            # sync queue: W1/W2 then edge slabs immediately; scalar queue:
            # dist/one-hot consts first, P3-only consts after (needed late)
            W1s = cload(W1, [H, H], BF16)
            W2s = cload(W2, [H, H], BF16)
            wdiags = cload(wdiag, [RMAX + 1, RMAX * 128], BF16, nc.scalar)
            distRs = cload(distR, [RMAX + 1, R * 128], BF16, nc.scalar)
            iotars = cload(iota_rep, [128, 128], BF16, nc.scalar)
            colps = cload(colp, [128, T_pad], BF16, nc.scalar)
            # P3-only consts: tiles reserved now, DMAs deferred into the
            # edge loop so early scalar-queue bandwidth goes to the consts
            # the first projections need
            Wress = cp.tile([H, H], BF16, tag="W_res")
            Wu1s = cp.tile([H, H], BF16, tag="Wu1")
            Wu2s = cp.tile([H, H], BF16, tag="Wu2")
            bupds = cp.tile([H, 1], F32, tag="bupd_col")
            emb_sb = bigp.tile([H, S_pad], BF16, tag="emb_sb")
            aggrT = bigp.tile([128, S_pad], BF16, tag="aggrT")

            def load_p3_consts():
                nc.scalar.dma_start(Wress[:], W_res[:])
                nc.scalar.dma_start(Wu1s[:], Wu1[:])
                nc.scalar.dma_start(Wu2s[:], Wu2[:])
                nc.scalar.dma_start(bupds[:], bupd_col[:])
                nc.scalar.dma_start(emb_sb[:], emb_shardT[:])

            # ---------- node update MLP, one 512-col block -----------------
            def emit_p3_block(b0):
                nb = min(512, S_pad - b0)
                ps_u = p3ps.tile([128, 512], F32, tag="p3ps", name="ps_u")
                pu = ps_u[:]
                nc.tensor.matmul(out=pu[:, 0:nb], lhsT=Wu1s[:],
                                 rhs=emb_sb[:, b0:b0 + nb], start=True,
                                 stop=False)
                nc.tensor.matmul(out=pu[:, 0:nb], lhsT=Wu2s[:],
                                 rhs=aggrT[:, b0:b0 + nb], start=False,
                                 stop=True)
                r_sb = sb.tile([128, 512], F32, tag="p3r", name="r_sb")
                nc.scalar.activation(out=r_sb[:, 0:nb], in_=pu[:, 0:nb],
                                     func=mybir.ActivationFunctionType.Relu,
                                     bias=bupds[:])
                ps_r = p3ps.tile([128, 512], F32, tag="p3ps", name="ps_r")
                pr = ps_r[:]
                nc.tensor.matmul(out=pr[:, 0:nb], lhsT=Wress[:],
                                 rhs=emb_sb[:, b0:b0 + nb], start=True,
                                 stop=True)
                o_sb = sb.tile([128, 512], F32, tag="p3o", name="o_sb")
                nc.vector.tensor_tensor(out=o_sb[:, 0:nb], in0=r_sb[:, 0:nb],
                                        in1=pr[:, 0:nb],
                                        op=mybir.AluOpType.add)
                nc.scalar.dma_start(out_d[:, b0:b0 + nb], o_sb[:, 0:nb])

            # window w's aggregate is final after its last tile; map final
            # tiles -> ready P3 blocks
            blocks_after = {}
            for b0 in range(0, S_pad, 512):
                wins = range(b0 // 128, min(b0 + 512, S_pad) // 128)
                fins = [last_wp[w] for w in wins if w in last_wp]
                if fins:
                    blocks_after.setdefault(max(fins), []).append(b0)
            p3_emitted = set()

            # zero windows that never receive edges, before any P3 block runs
            for w in range(n_win):
                if w not in first_wp:
                    nc.vector.memset(aggrT[:, w * 128:(w + 1) * 128], 0.0)

            # ---------- edge loop ------------------------------------------
            # segsum for run ri is emitted after run ri+1's projection
            # matmuls so the PE never stalls on relu (ACT) / o8 (DVE)
            aggr_tiles = {}
            copy_alt = [0]

            def emit_segsum(t0, L, msg_bf, o8):
                for k in range(L):
                    t = t0 + k
                    w = win_of_tile[t]
                    if t == first_wp[w]:
                        aggr_t = aggp.tile([128, H], F32, tag="aggr")
                        aggr_tiles[w] = aggr_t
                    nc.tensor.matmul(out=aggr_tiles[w][:],
                                     lhsT=msg_bf[:, k, :], rhs=o8[:, k, :],
                                     start=(t == first_wp[w]),
                                     stop=(t == last_wp[w]))
                    if t == last_wp[w]:
                        dstw = aggrT[:, w * 128:(w + 1) * 128]
                        if copy_alt[0] % 2:
                            nc.scalar.activation(
                                out=dstw, in_=aggr_tiles[w][:],
                                func=mybir.ActivationFunctionType.Copy)
                        else:
                            nc.vector.tensor_scalar_add(
                                out=dstw, in0=aggr_tiles[w][:], scalar1=0.0)
                        copy_alt[0] += 1
                        del aggr_tiles[w]
                        for b0 in blocks_after.get(t, []):
                            emit_p3_block(b0)
                            p3_emitted.add(b0)

            pending = None
            for ri in range(R):
                if ri == 1:
                    load_p3_consts()
                t0 = ri * RMAX
                L = min(RMAX, T - t0)

                src_sb = sb.tile([128, RMAX, H], BF16, tag="src", bufs=4)
                nc.sync.dma_start(src_sb[:, 0:L, :],
                                  srcT[:, t0 * 128:(t0 + L) * 128]
                                  .rearrange("p (k e) -> p k e", k=L))
                dst_sb = sb.tile([128, RMAX, H], BF16, tag="dst", bufs=4)
                nc.gpsimd.dma_start(dst_sb[:, 0:L, :],
                                    dstT[:, t0 * 128:(t0 + L) * 128]
                                    .rearrange("p (k e) -> p k e", k=L))

                msg_ps = ps.tile([128, RMAX, H], F32, tag="msgps")
                flat = msg_ps[:].rearrange("p k e -> p (k e)")
                # dist*w_d + b (rank-9, block-diagonal), opens accumulation
                for o in range(0, L * 128, 512):
                    oe = min(o + 512, L * 128)
                    nc.tensor.matmul(out=flat[:, o:oe],
                                     lhsT=distRs[:, ri * 128:(ri + 1) * 128],
                                     rhs=wdiags[:, o:oe], start=True,
                                     stop=False, skip_group_check=True)
                # + src @ W1 + dst @ W2 per tile
                for k in range(L):
                    nc.tensor.matmul(out=msg_ps[:, k, :],
                                     lhsT=src_sb[:, k, :], rhs=W1s[:],
                                     start=False, stop=False,
                                     skip_group_check=True)
                    nc.tensor.matmul(out=msg_ps[:, k, :],
                                     lhsT=dst_sb[:, k, :], rhs=W2s[:],
                                     start=False, stop=True,
                                     skip_group_check=True)
                # previous run's segment-sum (PE waits on its relu no more)
                if pending is not None:
                    emit_segsum(*pending)
                # relu + cast (ACT)
                msg_bf = sb.tile([128, RMAX, H], BF16, tag="msgb", bufs=3)
                nc.scalar.activation(out=msg_bf[:, 0:L, :],
                                     in_=msg_ps[:, 0:L, :],
                                     func=mybir.ActivationFunctionType.Relu)
                # scatter one-hot by local col
                o8 = sb.tile([128, RMAX, H], BF16, tag="o8", bufs=3)
                nc.vector.tensor_tensor(
                    out=o8[:, 0:L, :],
                    in0=colps[:, t0:t0 + L, None].to_broadcast([128, L, 128]),
                    in1=iotars[:, None, :].to_broadcast([128, L, 128]),
                    op=mybir.AluOpType.is_equal)
                pending = (t0, L, msg_bf, o8)

            if pending is not None:
                emit_segsum(*pending)
            for b0 in range(0, S_pad, 512):
                if b0 not in p3_emitted:
                    emit_p3_block(b0)

    nc.compile()
    return nc


# --------------------------------------------------------------------------
# entry point
# --------------------------------------------------------------------------

def kernel(node_embed, node_pos, W_res, W_msg, b_msg, W_upd, b_upd,
           edge_index, n_cores=8, _run=None):
    cfg, in_maps = host_prep(node_embed, node_pos, W_res, W_msg, b_msg,
                             W_upd, b_upd, edge_index, n_cores)
    nc = build_program(cfg)
    if _run is None:
        res = run_bass_kernel_spmd(nc, in_maps, core_ids=list(range(n_cores)))
        outs = [res.results[c]["out"] for c in range(n_cores)]
    else:
        outs = _run(nc, in_maps)
    return unshard(cfg, outs)


# revision 36
# speedup vs baseline: 1.9175x; 1.0525x over previous
"""Trainium2 Bass kernel for an equivariant GNN message-passing layer.

Full inputs in, full output out. 8-way owner-computes sharding by edge target
node (col). The host sorts each core's edges by target window, pads tiles to
128 lanes, and pre-gathers the raw endpoint embedding rows into sequential
slabs (srcT/dstT, transposed [ch, edge]); per-edge squared distances are
precomputed on host. The device computes, per core c (nodes [c*S, (c+1)*S)):

  msg[e]  = relu(src_e @ W1 + dst_e @ W2 + dist_e * w_d + b)   (f32 PSUM)
  aggrT   = one-hot scatter-sum of msg by col                  [128, S_pad]
  outT    = Wres^T emb^T + relu(Wu1^T emb^T + Wu2^T aggrT + b_upd)

with W1 = W_msg[:128], W2 = W_msg[128:256], w_d = W_msg[256]. All matmuls are
bf16 inputs with f32 PSUM accumulation. dist+bias enter via a single rank-9
matmul per 512 edge-columns (8 block-diagonal dist rows + a ones row streaming
[w_d blocks; b tiled]). The output is produced transposed [128, S_pad]; the
host transposes back.
"""

import sys

for _p in ("/opt/trn_rl_repo",):
    if _p not in sys.path:
        sys.path.insert(0, _p)

import numpy as np
import ml_dtypes

import concourse.bacc as bacc
import concourse.bass as bass
import concourse.mybir as mybir
import concourse.tile as tile
from concourse.bass_utils import run_bass_kernel_spmd

F32 = mybir.dt.float32
BF16 = mybir.dt.bfloat16
BF = ml_dtypes.bfloat16

H = 128          # hidden/in channels (hardcoded for this problem)
RMAX = 8         # tiles per run


# --------------------------------------------------------------------------
# host-side prep
# --------------------------------------------------------------------------

def host_prep(node_embed, node_pos, W_res, W_msg, b_msg, W_upd, b_upd,
              edge_index, n_cores):
    N, C_in = node_embed.shape
    assert C_in == H and W_msg.shape == (2 * H + 1, H)
    assert N % n_cores == 0
    S = N // n_cores
    n_win = -(-S // 128)
    S_pad = n_win * 128

    row = np.asarray(edge_index[0], dtype=np.int64)
    col = np.asarray(edge_index[1], dtype=np.int64)
    pos = np.asarray(node_pos, dtype=np.float32)
    diff = pos[row] - pos[col]
    dist = np.sum(diff * diff, axis=1).astype(np.float32)   # [E]

    # global 128-node blocks, assigned to (core, slot) by sorted edge count
    # round-robin so the per-slot max across cores (which sets the padded
    # tile count) tracks the mean instead of the tail
    NB = -(-N // 128)
    assert n_cores * n_win >= NB
    blk_of_edge = col // 128
    cnt_g = np.bincount(blk_of_edge, minlength=NB)
    order = np.argsort(-cnt_g, kind="stable")
    blk_at = np.full((n_cores, n_win), -1, dtype=np.int64)  # (c, slot) -> blk
    core_of_blk = np.zeros(NB, dtype=np.int64)
    slot_of_blk = np.zeros(NB, dtype=np.int64)
    for j in range(n_win):
        for c in range(n_cores):
            i = j * n_cores + c
            if i < NB:
                g = order[i]
                blk_at[c, j] = g
                core_of_blk[g] = c
                slot_of_blk[g] = j

    core_of = core_of_blk[blk_of_edge]

    # per-core edge lists sorted by slot
    per_core = []
    counts = np.zeros((n_cores, n_win), dtype=np.int64)
    for c in range(n_cores):
        sel = np.nonzero(core_of == c)[0]
        w = slot_of_blk[blk_of_edge[sel]]
        order_e = np.argsort(w, kind="stable")
        sel, w = sel[order_e], w[order_e]
        cw = (col[sel] % 128).astype(np.float32)
        np.add.at(counts[c], w, 1)
        per_core.append((sel, cw))

    tiles_w = -(-counts.max(axis=0) // 128)             # [n_win]
    win_of_tile = []
    for w in range(n_win):
        win_of_tile += [w] * int(tiles_w[w])
    T = len(win_of_tile)
    R = -(-T // RMAX)
    T_pad = R * RMAX
    first_wp, last_wp = {}, {}
    for t, w in enumerate(win_of_tile):
        first_wp.setdefault(w, t)
        last_wp[w] = t
    tile_base = {}
    b = 0
    for w in range(n_win):
        tile_base[w] = b
        b += int(tiles_w[w])

    embT = np.ascontiguousarray(np.asarray(node_embed, dtype=np.float32).T
                                ).astype(BF)            # [H, N]

    in_maps = []
    iota = np.arange(128, dtype=np.float32)
    W_msg = np.asarray(W_msg, dtype=np.float32)
    W_upd = np.asarray(W_upd, dtype=np.float32)
    # rank-9 dist+bias rhs: rows 0..7 block-diagonal w_d, row 8 = b tiled
    wdiag = np.zeros((RMAX + 1, RMAX * 128), dtype=np.float32)
    for k in range(RMAX):
        wdiag[k, k * 128:(k + 1) * 128] = W_msg[2 * H]
        wdiag[RMAX, k * 128:(k + 1) * 128] = np.asarray(b_msg, np.float32)
    repl = {
        "W1": np.ascontiguousarray(W_msg[:H]).astype(BF),
        "W2": np.ascontiguousarray(W_msg[H:2 * H]).astype(BF),
        "wdiag": wdiag.astype(BF),
        "W_res": np.asarray(W_res, dtype=np.float32).astype(BF),
        "Wu1": np.ascontiguousarray(W_upd[:H]).astype(BF),
        "Wu2": np.ascontiguousarray(W_upd[H:]).astype(BF),
        "bupd_col": np.asarray(b_upd, dtype=np.float32).reshape(H, 1),
        "iota_rep": np.tile(iota.reshape(1, 128), (128, 1)).astype(BF),
    }

    emb = np.asarray(node_embed, dtype=np.float32)
    for c in range(n_cores):
        sel, cw = per_core[c]
        # per-tile edge slot assignment (window-major, padded per window)
        rows_pad = np.zeros(T_pad * 128, dtype=np.int64)
        cols_pad = np.zeros(T_pad * 128, dtype=np.int64)
        valid = np.zeros(T_pad * 128, dtype=bool)
        colp = np.full((128, T_pad), -1.0, dtype=np.float32)
        distp = np.zeros((T_pad, 128), dtype=np.float32)
        start = 0
        for w in range(n_win):
            cnt = int(counts[c, w])
            if cnt:
                idx = np.arange(cnt)
                slot = (tile_base[w] + idx // 128) * 128 + idx % 128
                e = sel[start:start + cnt]
                rows_pad[slot] = row[e]
                cols_pad[slot] = col[e]
                valid[slot] = True
                distp.reshape(-1)[slot] = dist[e]
                colp[idx % 128, tile_base[w] + idx // 128] = cw[start:start + cnt]
                start += cnt
        # pre-gathered transposed slabs [ch, T_pad*128]
        srcT = embT[:, rows_pad].copy()
        dstT = embT[:, cols_pad].copy()
        srcT[:, ~valid] = 0
        dstT[:, ~valid] = 0
        # distR: per run [9, 128]: rows 0..7 = tile dists, row 8 = ones
        distR = np.zeros((RMAX + 1, R * 128), dtype=np.float32)
        dr = distp.reshape(R, RMAX, 128)
        for k in range(RMAX):
            distR[k] = dr[:, k, :].reshape(R * 128)
        distR[RMAX] = 1.0
        m = dict(repl)
        shardT = np.zeros((H, S_pad), dtype=BF)
        for j in range(n_win):
            g = blk_at[c, j]
            if g >= 0:
                nb = min(128, N - g * 128)
                shardT[:, j * 128:j * 128 + nb] = \
                    emb[g * 128:g * 128 + nb].T.astype(BF)
        m["emb_shardT"] = shardT
        m["srcT"] = srcT
        m["dstT"] = dstT
        m["distR"] = distR.astype(BF)
        m["colp"] = colp.astype(BF)
        in_maps.append(m)

    cfg = dict(N=N, S=S, S_pad=S_pad, n_win=n_win, R=R, T=T, T_pad=T_pad,
               win_of_tile=win_of_tile, first_wp=first_wp, last_wp=last_wp,
               n_cores=n_cores, blk_at=blk_at)
    return cfg, in_maps


def unshard(cfg, outs):
    """Assemble the full [N, H] output from per-core [H, S_pad] transposed
    slabs laid out in (core, slot) block order."""
    N, n_win, n_cores = cfg["N"], cfg["n_win"], cfg["n_cores"]
    blk_at = cfg["blk_at"]
    out = np.empty((N, H), dtype=np.float32)
    for c in range(n_cores):
        for j in range(n_win):
            g = blk_at[c, j]
            if g >= 0:
                nb = min(128, N - g * 128)
                out[g * 128:g * 128 + nb] = \
                    outs[c][:, j * 128:j * 128 + nb].T
    return out


# --------------------------------------------------------------------------
# device program
# --------------------------------------------------------------------------

def build_program(cfg, debug=False):
    S_pad, n_win, R, T, T_pad = (cfg["S_pad"], cfg["n_win"], cfg["R"],
                                 cfg["T"], cfg["T_pad"])
    win_of_tile = cfg["win_of_tile"]
    first_wp, last_wp = cfg["first_wp"], cfg["last_wp"]

    nc = bacc.Bacc("TRN2", target_bir_lowering=False, debug=debug,
                   num_devices=cfg["n_cores"])

    din = lambda n, s, dt: nc.dram_tensor(n, s, dt, kind="ExternalInput")
    W1 = din("W1", [H, H], BF16)
    W2 = din("W2", [H, H], BF16)
    wdiag = din("wdiag", [RMAX + 1, RMAX * 128], BF16)
    W_res = din("W_res", [H, H], BF16)
    Wu1 = din("Wu1", [H, H], BF16)
    Wu2 = din("Wu2", [H, H], BF16)
    bupd_col = din("bupd_col", [H, 1], F32)
    iota_rep = din("iota_rep", [128, 128], BF16)
    emb_shardT = din("emb_shardT", [H, S_pad], BF16)
    srcT = din("srcT", [H, T_pad * 128], BF16)
    dstT = din("dstT", [H, T_pad * 128], BF16)
    distR = din("distR", [RMAX + 1, R * 128], BF16)
    colp = din("colp", [128, T_pad], BF16)

    out_d = nc.dram_tensor("out", [H, S_pad], F32, kind="ExternalOutput")

    with tile.TileContext(nc) as tc:
        with (
            tc.tile_pool(name="const", bufs=1) as cp,
            tc.tile_pool(name="sb", bufs=2) as sb,
            tc.tile_pool(name="big", bufs=1) as bigp,
            tc.tile_pool(name="ps", bufs=2, space="PSUM") as ps,
            tc.tile_pool(name="aggp", bufs=2, space="PSUM") as aggp,
            tc.tile_pool(name="p3ps", bufs=2, space="PSUM") as p3ps,
        ):
            def cload(t, shape, dt, eng=None):
                s = cp.tile(shape, dt, tag=t.name)
                (eng or nc.sync).dma_start(s[:], t[:])
                return s

            # sync queue: W1/W2 then edge slabs immediately; scalar queue:
            # dist/one-hot consts first, P3-only consts after (needed late)
            W1s = cload(W1, [H, H], BF16)
            W2s = cload(W2, [H, H], BF16)
            wdiags = cload(wdiag, [RMAX + 1, RMAX * 128], BF16, nc.scalar)
            # chunk the two big edge consts: a small head unblocks the first
            # runs ~10us earlier; the tails stream in behind the first slabs
            head = 16 * 128
            distRs = cp.tile([RMAX + 1, R * 128], BF16, tag="distR")
            nc.scalar.dma_start(distRs[:, 0:head], distR[:, 0:head])
            iotars = cload(iota_rep, [128, 128], BF16, nc.scalar)
            colps = cp.tile([128, T_pad], BF16, tag="colp")
            nc.scalar.dma_start(colps[:, 0:128], colp[:, 0:128])
            nc.scalar.dma_start(distRs[:, head:], distR[:, head:])
            nc.scalar.dma_start(colps[:, 128:], colp[:, 128:])
            # P3-only consts: tiles reserved now, DMAs deferred into the
            # edge loop so early scalar-queue bandwidth goes to the consts
            # the first projections need
            Wress = cp.tile([H, H], BF16, tag="W_res")
            Wu1s = cp.tile([H, H], BF16, tag="Wu1")
            Wu2s = cp.tile([H, H], BF16, tag="Wu2")
            bupds = cp.tile([H, 1], F32, tag="bupd_col")
            emb_sb = bigp.tile([H, S_pad], BF16, tag="emb_sb")
            aggrT = bigp.tile([128, S_pad], BF16, tag="aggrT")

            def load_p3_consts():
                nc.scalar.dma_start(Wress[:], W_res[:])
                nc.scalar.dma_start(Wu1s[:], Wu1[:])
                nc.scalar.dma_start(Wu2s[:], Wu2[:])
                nc.scalar.dma_start(bupds[:], bupd_col[:])
                nc.scalar.dma_start(emb_sb[:], emb_shardT[:])

            # ---------- node update MLP, one 512-col block -----------------
            def emit_p3_block(b0):
                nb = min(512, S_pad - b0)
                ps_u = p3ps.tile([128, 512], F32, tag="p3ps", name="ps_u")
                pu = ps_u[:]
                nc.tensor.matmul(out=pu[:, 0:nb], lhsT=Wu1s[:],
                                 rhs=emb_sb[:, b0:b0 + nb], start=True,
                                 stop=False)
                nc.tensor.matmul(out=pu[:, 0:nb], lhsT=Wu2s[:],
                                 rhs=aggrT[:, b0:b0 + nb], start=False,
                                 stop=True)
                r_sb = sb.tile([128, 512], F32, tag="p3r", name="r_sb")
                nc.scalar.activation(out=r_sb[:, 0:nb], in_=pu[:, 0:nb],
                                     func=mybir.ActivationFunctionType.Relu,
                                     bias=bupds[:])
                ps_r = p3ps.tile([128, 512], F32, tag="p3ps", name="ps_r")
                pr = ps_r[:]
                nc.tensor.matmul(out=pr[:, 0:nb], lhsT=Wress[:],
                                 rhs=emb_sb[:, b0:b0 + nb], start=True,
                                 stop=True)
                o_sb = sb.tile([128, 512], F32, tag="p3o", name="o_sb")
                nc.vector.tensor_tensor(out=o_sb[:, 0:nb], in0=r_sb[:, 0:nb],
                                        in1=pr[:, 0:nb],
                                        op=mybir.AluOpType.add)
                nc.scalar.dma_start(out_d[:, b0:b0 + nb], o_sb[:, 0:nb])

            # window w's aggregate is final after its last tile; map final
            # tiles -> ready P3 blocks
            blocks_after = {}
            for b0 in range(0, S_pad, 512):
                wins = range(b0 // 128, min(b0 + 512, S_pad) // 128)
                fins = [last_wp[w] for w in wins if w in last_wp]
                if fins:
                    blocks_after.setdefault(max(fins), []).append(b0)
            p3_emitted = set()

            # zero windows that never receive edges, before any P3 block runs
            for w in range(n_win):
                if w not in first_wp:
                    nc.vector.memset(aggrT[:, w * 128:(w + 1) * 128], 0.0)

            # ---------- edge loop ------------------------------------------
            # segsum for run ri is emitted after run ri+1's projection
            # matmuls so the PE never stalls on relu (ACT) / o8 (DVE)
            aggr_tiles = {}
            copy_alt = [0]

            def emit_segsum(t0, L, msg_bf, o8):
                for k in range(L):
                    t = t0 + k
                    w = win_of_tile[t]
                    if t == first_wp[w]:
                        aggr_t = aggp.tile([128, H], F32, tag="aggr")
                        aggr_tiles[w] = aggr_t
                    nc.tensor.matmul(out=aggr_tiles[w][:],
                                     lhsT=msg_bf[:, k, :], rhs=o8[:, k, :],
                                     start=(t == first_wp[w]),
                                     stop=(t == last_wp[w]))
                    if t == last_wp[w]:
                        dstw = aggrT[:, w * 128:(w + 1) * 128]
                        if copy_alt[0] % 2:
                            nc.scalar.activation(
                                out=dstw, in_=aggr_tiles[w][:],
                                func=mybir.ActivationFunctionType.Copy)
                        else:
                            nc.vector.tensor_scalar_add(
                                out=dstw, in0=aggr_tiles[w][:], scalar1=0.0)
                        copy_alt[0] += 1
                        del aggr_tiles[w]
                        for b0 in blocks_after.get(t, []):
                            emit_p3_block(b0)
                            p3_emitted.add(b0)

            pending = None
            for ri in range(R):
                if ri == 1:
                    load_p3_consts()
                t0 = ri * RMAX
                L = min(RMAX, T - t0)

                src_sb = sb.tile([128, RMAX, H], BF16, tag="src", bufs=4)
                nc.sync.dma_start(src_sb[:, 0:L, :],
                                  srcT[:, t0 * 128:(t0 + L) * 128]
                                  .rearrange("p (k e) -> p k e", k=L))
                dst_sb = sb.tile([128, RMAX, H], BF16, tag="dst", bufs=4)
                nc.gpsimd.dma_start(dst_sb[:, 0:L, :],
                                    dstT[:, t0 * 128:(t0 + L) * 128]
                                    .rearrange("p (k e) -> p k e", k=L))

                msg_ps = ps.tile([128, RMAX, H], F32, tag="msgps")
                flat = msg_ps[:].rearrange("p k e -> p (k e)")
                # dist*w_d + b (rank-9, block-diagonal), opens accumulation
                for o in range(0, L * 128, 512):
                    oe = min(o + 512, L * 128)
                    nc.tensor.matmul(out=flat[:, o:oe],
                                     lhsT=distRs[:, ri * 128:(ri + 1) * 128],
                                     rhs=wdiags[:, o:oe], start=True,
                                     stop=False, skip_group_check=True)
                # + src @ W1 + dst @ W2 per tile
                for k in range(L):
                    nc.tensor.matmul(out=msg_ps[:, k, :],
                                     lhsT=src_sb[:, k, :], rhs=W1s[:],
                                     start=False, stop=False,
                                     skip_group_check=True)
                    nc.tensor.matmul(out=msg_ps[:, k, :],
                                     lhsT=dst_sb[:, k, :], rhs=W2s[:],
                                     start=False, stop=True,
                                     skip_group_check=True)
                # previous run's segment-sum (PE waits on its relu no more)
                if pending is not None:
                    emit_segsum(*pending)
                # relu + cast (ACT)
                msg_bf = sb.tile([128, RMAX, H], BF16, tag="msgb", bufs=3)
                nc.scalar.activation(out=msg_bf[:, 0:L, :],
                                     in_=msg_ps[:, 0:L, :],
                                     func=mybir.ActivationFunctionType.Relu)
                # scatter one-hot by local col
                o8 = sb.tile([128, RMAX, H], BF16, tag="o8", bufs=3)
                nc.vector.tensor_tensor(
                    out=o8[:, 0:L, :],
                    in0=colps[:, t0:t0 + L, None].to_broadcast([128, L, 128]),
                    in1=iotars[:, None, :].to_broadcast([128, L, 128]),
                    op=mybir.AluOpType.is_equal)
                pending = (t0, L, msg_bf, o8)

            if pending is not None:
                emit_segsum(*pending)
            for b0 in range(0, S_pad, 512):
                if b0 not in p3_emitted:
                    emit_p3_block(b0)

    nc.compile()
    return nc


# --------------------------------------------------------------------------
# entry point
# --------------------------------------------------------------------------

def kernel(node_embed, node_pos, W_res, W_msg, b_msg, W_upd, b_upd,
           edge_index, n_cores=8, _run=None):
    cfg, in_maps = host_prep(node_embed, node_pos, W_res, W_msg, b_msg,
                             W_upd, b_upd, edge_index, n_cores)
    nc = build_program(cfg)
    if _run is None:
        res = run_bass_kernel_spmd(nc, in_maps, core_ids=list(range(n_cores)))
        outs = [res.results[c]["out"] for c in range(n_cores)]
    else:
        outs = _run(nc, in_maps)
    return unshard(cfg, outs)
